# revision 25
# baseline (speedup 1.0000x reference)
"""Trainium2 Bass kernel for nn_CIntegration_3487513444382 (embedding_lookup).

Computation (per token): ct = concat(onehot(rgap,32), onehot(sgap,32),
onehot(pcount,32)); out = concat(vt * (ct @ W.T), ct).

Strategy: pure data parallel over the batch dim (64 -> 8 per core), with
all device-side tensors transposed to [feature, token] so the rel-err
budget (2e-2) can buy bandwidth: vt is fed as bf16, theta is stored as
bf16, and the one-hot tail is stored as fp8 via a casting SWDGE DMA
(0/1 are exact in fp8). Per core this moves ~9.3MB instead of ~20MB.

Per 1024-token pair of groups: a zero-padded E3 matmul (K=96 so the PE
activity monitor sees a busy stream and can reach 2.4GHz) broadcasts
the offset indices to 96 partitions, one FD-1024 DVE compare against an
iota column builds the transposed one-hot in bf16, the PE streams it
through stationary W.T halves into two 2-bank PSUM tiles, ScalarE
copies them to a pair-wide SBUF tile as bf16 (FD-1024 each), and one
FD-2048 2x-mode DVE multiply applies the vt gate. The issue order is
software-pipelined a full pair ahead (compares lead the PE) so no
engine waits on another in steady state; all vt loads are issued up
front so the late stream window belongs to the stores.
"""
import numpy as np

import concourse.bass as bass
import concourse.tile as tile
from concourse import bacc, mybir
from concourse.bass_utils import run_bass_kernel_spmd

F32 = mybir.dt.float32
BF16 = mybir.dt.bfloat16
FP8 = mybir.dt.float8e4

N_CORES = 8
B, S, E = 64, 1024, 256
BPC = B // N_CORES          # 8 batches per core
NTOK = BPC * S              # 8192 tokens per core
NCH = NTOK // 128           # 64 chunks of 128 tokens
GTOK = 512                  # tokens per compute group
PTOK = 2 * GTOK             # tokens per pair (DMA batch)
NPAIR = NTOK // PTOK        # 8
NTOT = 96                   # one-hot width
EH = E // 128               # 2 e-halves

_NC = None


def _build_nc():
    nc = bacc.Bacc("TRN2", target_bir_lowering=False, debug=False,
                   num_devices=N_CORES)
    vtT = nc.dram_tensor("vtT", [E, NTOK], BF16, kind="ExternalInput")
    idxt = nc.dram_tensor("idxt", [3, NTOK], BF16, kind="ExternalInput")
    wt = nc.dram_tensor("wt", [NTOT, E], BF16, kind="ExternalInput")
    thetaT = nc.dram_tensor("thetaT", [E, NTOK], BF16, kind="ExternalOutput")
    ctT = nc.dram_tensor("ctT", [NTOT, NTOK], FP8, kind="ExternalOutput")

    with tile.TileContext(nc) as tc:
        with (
            tc.tile_pool(name="const", bufs=1) as const,
            tc.tile_pool(name="vtp", bufs=8) as vtp,
            tc.tile_pool(name="outp", bufs=3) as outp,
            tc.tile_pool(name="ctp", bufs=4) as ctp,
            tc.tile_pool(name="mmsb", bufs=3) as mmsb,
            tc.tile_pool(name="ps_b", bufs=2, space="PSUM") as ps_b,
            tc.tile_pool(name="ps_m", bufs=2, space="PSUM") as ps_m,
        ):
            # [e, tok] views split the 256 e-rows into 2 x 128 partitions
            vt_view = vtT.ap().rearrange("(h p) t -> p h t", p=128)
            th_view = thetaT.ap().rearrange("(h p) t -> p h t", p=128)
            # real index rows live on partitions 93..95 of a 96-row tile;
            # rows 0..92 are zeroed so the bc matmul streams 96 active
            # rows (a K=3 stream reads as "idle" to the PE activity
            # monitor and blocks the 2.4GHz up-shift)
            idxt_sb = const.tile([NTOT, NTOK], BF16)
            nc.sync.dma_start(idxt_sb[NTOT - 3:, 0:PTOK],
                              idxt.ap()[:, 0:PTOK])
            # first vt pair next on the same ring
            vt0 = vtp.tile([128, EH, PTOK], BF16, tag="vt")
            nc.sync.dma_start(vt0[:], vt_view[:, :, 0:PTOK])
            nc.sync.dma_start(idxt_sb[NTOT - 3:, PTOK:], idxt.ap()[:, PTOK:])
            # zero-fill the 93 padding rows on the Scalar engine, whose
            # pipe is idle until the first Cct copy; integer mul-by-0
            # path, so SBUF junk (even NaN patterns) is safe. Split so
            # the head (pairs 0-1) unblocks the first bc matmul.
            nc.scalar.memzero(idxt_sb[0:NTOT - 3, 0:2 * PTOK])
            nc.scalar.memzero(idxt_sb[0:NTOT - 3, 2 * PTOK:])
            # weight on the independent ACT HWDGE ring
            wt_sb = const.tile([NTOT, E], BF16)
            nc.scalar.dma_start(wt_sb[:], wt.ap())
            # small device-built constants on GpSimd, ahead of its
            # SWDGE descriptor burst
            e3_sb = const.tile([NTOT, NTOT], BF16)
            nc.gpsimd.memset(e3_sb[:], 1.0)
            nc.gpsimd.affine_select(
                out=e3_sb[:].rearrange("p (a b) -> p a b", a=3),
                in_=e3_sb[:].rearrange("p (a b) -> p a b", a=3),
                pattern=[[1, 3], [0, 32]],
                compare_op=mybir.AluOpType.is_equal,
                fill=0.0, base=NTOT - 3, channel_multiplier=-1,
            )
            iota_col = const.tile([NTOT, 1], F32)
            nc.gpsimd.iota(iota_col[:], [[0, 1]], channel_multiplier=1,
                           allow_small_or_imprecise_dtypes=True)
            # all remaining vt pairs up front on SWDGE: loads drain early,
            # leaving the late stream window to the stores
            vt_tiles = [vt0]
            for p in range(1, NPAIR):
                t = vtp.tile([128, EH, PTOK], BF16, tag="vt")
                nc.gpsimd.dma_start(
                    t[:], vt_view[:, :, p * PTOK:(p + 1) * PTOK])
                vt_tiles.append(t)

            def bc_mm(p):
                # one PSUM tile per pair; matmul output must fit a single
                # 2KB bank, so two 512-col matmuls fill its halves
                t = ps_b.tile([NTOT, PTOK], F32, tag="bc")
                for g in range(2):
                    lo = p * PTOK + g * GTOK
                    nc.tensor.matmul(
                        t[:, g * GTOK:(g + 1) * GTOK], e3_sb[:],
                        idxt_sb[:, lo:lo + GTOK],
                        start=True, stop=True,
                    )
                return t

            def compare(p, bc_t):
                # transposed one-hot for the whole pair in one FD-1024 op
                t = ctp.tile([NTOT, PTOK], BF16, tag="ct_t",
                             name=f"ct_t_{p}")
                nc.vector.tensor_scalar(
                    t[:], bc_t[:], iota_col[:, 0:1], None,
                    mybir.AluOpType.is_equal,
                )
                return t

            # software-pipeline prologue: one full pair of lookahead
            bc_tiles = {0: bc_mm(0), 1: bc_mm(1)}
            ct_tiles = {0: compare(0, bc_tiles.pop(0))}

            for p in range(NPAIR):
                # next pair's one-hot first: by the time the PE reaches
                # quad(p+1) it never waits on the DVE
                if p + 1 < NPAIR:
                    ct_tiles[p + 1] = compare(p + 1, bc_tiles.pop(p + 1))
                ct_t = ct_tiles.pop(p)
                # Cct.T: 4 matmuls, same-stationary adjacent, into two
                # 2-bank group tiles
                mm_ps = [ps_m.tile([128, EH, GTOK], F32, tag="mm",
                                   name=f"mm_{p}_{g}") for g in range(2)]
                for h in range(EH):
                    for g in range(2):
                        nc.tensor.matmul(
                            mm_ps[g][:, h, :],
                            wt_sb[:, h * 128:(h + 1) * 128],
                            ct_t[:, g * GTOK:(g + 1) * GTOK],
                            start=True, stop=True,
                        )
                # keep the PE fed: the bc for two pairs out
                if p + 2 < NPAIR:
                    bc_tiles[p + 2] = bc_mm(p + 2)
                # ct store with bf16->fp8 cast done by the DMA itself
                # (SWDGE-only feature): zero engine cost, fp8 HBM bytes
                nc.gpsimd.dma_start(
                    ctT.ap()[:, p * PTOK:(p + 1) * PTOK], ct_t[:])

                vt_big = vt_tiles[p]
                th_tile = outp.tile([128, EH, PTOK], BF16)
                # PSUM -> SBUF bf16 on the Scalar engine so the gate runs
                # in DVE 2x mode; pair-wide mm_sb so the gate is a single
                # FD-2048 op
                mm_sb = mmsb.tile([128, EH, PTOK], BF16, name=f"mm_sb_{p}")
                for g in range(2):
                    nc.scalar.copy(
                        mm_sb[:, :, g * GTOK:(g + 1) * GTOK], mm_ps[g][:])
                if p < NPAIR - 1:
                    nc.vector.tensor_tensor(
                        th_tile[:], vt_big[:], mm_sb[:],
                        mybir.AluOpType.mult,
                    )
                    nc.sync.dma_start(
                        th_view[:, :, p * PTOK:(p + 1) * PTOK], th_tile[:])
                else:
                    # endgame: gate + store per group so the final drain
                    # is half-sized and starts right after its gate
                    for g in range(2):
                        sl = slice(g * GTOK, (g + 1) * GTOK)
                        nc.vector.tensor_tensor(
                            th_tile[:, :, sl], vt_big[:, :, sl],
                            mm_sb[:, :, sl],
                            mybir.AluOpType.mult,
                        )
                        lo = p * PTOK + g * GTOK
                        nc.sync.dma_start(
                            th_view[:, :, lo:lo + GTOK],
                            th_tile[:, :, sl])

    nc.compile()
    return nc


def _get_nc():
    global _NC
    if _NC is None:
        _NC = _build_nc()
    return _NC


def _host_prep(vt, rgap, sgap, pcount, W):
    import ml_dtypes
    bf16 = ml_dtypes.bfloat16
    vt = np.asarray(vt, dtype=np.float32)
    rgap = np.asarray(rgap)
    sgap = np.asarray(sgap)
    pcount = np.asarray(pcount)
    W = np.asarray(W, dtype=np.float32)
    wt = np.ascontiguousarray(W.T).astype(bf16)     # [96, 256]
    in_maps = []
    for m in range(N_CORES):
        sl = slice(m * BPC, (m + 1) * BPC)
        # token t = p*64 + i maps to column tau = i*128 + p
        idxs = np.stack(
            [rgap[sl].reshape(NTOK),
             sgap[sl].reshape(NTOK) + 32,
             pcount[sl].reshape(NTOK) + 64], axis=0)          # [3, t]
        idxt = np.ascontiguousarray(
            idxs.reshape(3, 128, NCH).transpose(0, 2, 1).reshape(3, NTOK)
        ).astype(bf16)
        vtT = np.ascontiguousarray(
            vt[sl].reshape(128, NCH, E).transpose(2, 1, 0).reshape(E, NTOK)
        ).astype(bf16)
        in_maps.append({"vtT": vtT, "idxt": idxt, "wt": wt})
    return in_maps


def kernel(vt, rgap, sgap, pcount, W, _trace=False, _tmpdir=None):
    nc = _get_nc()
    in_maps = _host_prep(vt, rgap, sgap, pcount, W)
    res = run_bass_kernel_spmd(
        nc, in_maps, list(range(N_CORES)),
        trace=_trace, **({"tmpdir": _tmpdir} if _tmpdir else {}),
    )
    full = np.empty((B, S, E + NTOT), dtype=np.float32)
    for m in range(N_CORES):
        sl = slice(m * BPC, (m + 1) * BPC)
        view = full[sl].reshape(NTOK, E + NTOT)
        thetaT = np.asarray(res.results[m]["thetaT"]).astype(np.float32)
        ct8 = np.asarray(res.results[m]["ctT"]).astype(np.float32)
        view[:, :E] = thetaT.reshape(E, NCH, 128).transpose(2, 1, 0) \
                            .reshape(NTOK, E)
        view[:, E:] = ct8.reshape(NTOT, NCH, 128).transpose(2, 1, 0) \
                         .reshape(NTOK, NTOT)
    if _trace:
        return full, res
    return full


# revision 27
# speedup vs baseline: 1.0230x; 1.0230x over previous
"""Trainium2 Bass kernel for nn_CIntegration_3487513444382 (embedding_lookup).

Computation (per token): ct = concat(onehot(rgap,32), onehot(sgap,32),
onehot(pcount,32)); out = concat(vt * (ct @ W.T), ct).

Strategy: pure data parallel over the batch dim (64 -> 8 per core), with
all device-side tensors transposed to [feature, token] so the rel-err
budget (2e-2) can buy bandwidth: vt is fed as bf16, theta is stored as
bf16, and the one-hot tail is stored as fp8 via a casting SWDGE DMA
(0/1 are exact in fp8). Per core this moves ~9.3MB instead of ~20MB.

Per 1024-token pair of groups: a zero-padded E3 matmul (K=96 so the PE
activity monitor sees a busy stream and can reach 2.4GHz) broadcasts
the offset indices to 96 partitions, one FD-1024 DVE compare against an
iota column builds the transposed one-hot in bf16, the PE streams it
through stationary W.T halves into two 2-bank PSUM tiles, ScalarE
copies each to SBUF as bf16 (FD-1024), and per-group 2x-mode DVE
multiplies apply the vt gate. The issue order is
software-pipelined a full pair ahead (compares lead the PE) so no
engine waits on another in steady state; all vt loads are issued up
front so the late stream window belongs to the stores.
"""
import numpy as np

import concourse.bass as bass
import concourse.tile as tile
from concourse import bacc, mybir
from concourse.bass_utils import run_bass_kernel_spmd

F32 = mybir.dt.float32
BF16 = mybir.dt.bfloat16
FP8 = mybir.dt.float8e4

N_CORES = 8
B, S, E = 64, 1024, 256
BPC = B // N_CORES          # 8 batches per core
NTOK = BPC * S              # 8192 tokens per core
NCH = NTOK // 128           # 64 chunks of 128 tokens
GTOK = 512                  # tokens per compute group
PTOK = 2 * GTOK             # tokens per pair (DMA batch)
NPAIR = NTOK // PTOK        # 8
NTOT = 96                   # one-hot width
EH = E // 128               # 2 e-halves

_NC = None


def _build_nc():
    nc = bacc.Bacc("TRN2", target_bir_lowering=False, debug=False,
                   num_devices=N_CORES)
    vtT = nc.dram_tensor("vtT", [E, NTOK], BF16, kind="ExternalInput")
    idxt = nc.dram_tensor("idxt", [3, NTOK], BF16, kind="ExternalInput")
    wt = nc.dram_tensor("wt", [NTOT, E], BF16, kind="ExternalInput")
    thetaT = nc.dram_tensor("thetaT", [E, NTOK], BF16, kind="ExternalOutput")
    ctT = nc.dram_tensor("ctT", [NTOT, NTOK], FP8, kind="ExternalOutput")

    with tile.TileContext(nc) as tc:
        with (
            tc.tile_pool(name="const", bufs=1) as const,
            tc.tile_pool(name="vtp", bufs=8) as vtp,
            tc.tile_pool(name="outp", bufs=3) as outp,
            tc.tile_pool(name="ctp", bufs=4) as ctp,
            tc.tile_pool(name="mmsb", bufs=3) as mmsb,
            tc.tile_pool(name="ps_b", bufs=2, space="PSUM") as ps_b,
            tc.tile_pool(name="ps_m", bufs=2, space="PSUM") as ps_m,
        ):
            # [e, tok] views split the 256 e-rows into 2 x 128 partitions
            vt_view = vtT.ap().rearrange("(h p) t -> p h t", p=128)
            th_view = thetaT.ap().rearrange("(h p) t -> p h t", p=128)
            # real index rows live on partitions 93..95 of a 96-row tile;
            # rows 0..92 are zeroed so the bc matmul streams 96 active
            # rows (a K=3 stream reads as "idle" to the PE activity
            # monitor and blocks the 2.4GHz up-shift)
            idxt_sb = const.tile([NTOT, NTOK], BF16)
            nc.sync.dma_start(idxt_sb[NTOT - 3:, 0:PTOK],
                              idxt.ap()[:, 0:PTOK])
            # first vt pair next on the same ring
            vt0 = vtp.tile([128, EH, PTOK], BF16, tag="vt")
            nc.sync.dma_start(vt0[:], vt_view[:, :, 0:PTOK])
            nc.sync.dma_start(idxt_sb[NTOT - 3:, PTOK:], idxt.ap()[:, PTOK:])
            # zero-fill the 93 padding rows on the Scalar engine, whose
            # pipe is idle until the first Cct copy; integer mul-by-0
            # path, so SBUF junk (even NaN patterns) is safe. Split so
            # the head (pairs 0-1) unblocks the first bc matmul.
            nc.scalar.memzero(idxt_sb[0:NTOT - 3, 0:2 * PTOK])
            nc.scalar.memzero(idxt_sb[0:NTOT - 3, 2 * PTOK:])
            # weight on the independent ACT HWDGE ring
            wt_sb = const.tile([NTOT, E], BF16)
            nc.scalar.dma_start(wt_sb[:], wt.ap())
            # small device-built constants on GpSimd, ahead of its
            # SWDGE descriptor burst
            e3_sb = const.tile([NTOT, NTOT], BF16)
            nc.gpsimd.memset(e3_sb[:], 1.0)
            nc.gpsimd.affine_select(
                out=e3_sb[:].rearrange("p (a b) -> p a b", a=3),
                in_=e3_sb[:].rearrange("p (a b) -> p a b", a=3),
                pattern=[[1, 3], [0, 32]],
                compare_op=mybir.AluOpType.is_equal,
                fill=0.0, base=NTOT - 3, channel_multiplier=-1,
            )
            iota_col = const.tile([NTOT, 1], F32)
            nc.gpsimd.iota(iota_col[:], [[0, 1]], channel_multiplier=1,
                           allow_small_or_imprecise_dtypes=True)
            # all remaining vt pairs up front on SWDGE: loads drain early,
            # leaving the late stream window to the stores
            vt_tiles = [vt0]
            for p in range(1, NPAIR):
                t = vtp.tile([128, EH, PTOK], BF16, tag="vt")
                nc.gpsimd.dma_start(
                    t[:], vt_view[:, :, p * PTOK:(p + 1) * PTOK])
                vt_tiles.append(t)

            def bc_mm(p):
                # one PSUM tile per pair; matmul output must fit a single
                # 2KB bank, so two 512-col matmuls fill its halves
                t = ps_b.tile([NTOT, PTOK], F32, tag="bc")
                for g in range(2):
                    lo = p * PTOK + g * GTOK
                    nc.tensor.matmul(
                        t[:, g * GTOK:(g + 1) * GTOK], e3_sb[:],
                        idxt_sb[:, lo:lo + GTOK],
                        start=True, stop=True,
                    )
                return t

            def compare(p, bc_t):
                # transposed one-hot for the whole pair in one FD-1024 op
                t = ctp.tile([NTOT, PTOK], BF16, tag="ct_t",
                             name=f"ct_t_{p}")
                nc.vector.tensor_scalar(
                    t[:], bc_t[:], iota_col[:, 0:1], None,
                    mybir.AluOpType.is_equal,
                )
                return t

            # software-pipeline prologue: one full pair of lookahead
            bc_tiles = {0: bc_mm(0), 1: bc_mm(1)}
            ct_tiles = {0: compare(0, bc_tiles.pop(0))}

            for p in range(NPAIR):
                # next pair's one-hot first: by the time the PE reaches
                # quad(p+1) it never waits on the DVE
                if p + 1 < NPAIR:
                    ct_tiles[p + 1] = compare(p + 1, bc_tiles.pop(p + 1))
                ct_t = ct_tiles.pop(p)
                # Cct.T: 4 matmuls, same-stationary adjacent, into two
                # 2-bank group tiles
                mm_ps = [ps_m.tile([128, EH, GTOK], F32, tag="mm",
                                   name=f"mm_{p}_{g}") for g in range(2)]
                for h in range(EH):
                    for g in range(2):
                        nc.tensor.matmul(
                            mm_ps[g][:, h, :],
                            wt_sb[:, h * 128:(h + 1) * 128],
                            ct_t[:, g * GTOK:(g + 1) * GTOK],
                            start=True, stop=True,
                        )
                # keep the PE fed: the bc for two pairs out
                if p + 2 < NPAIR:
                    bc_tiles[p + 2] = bc_mm(p + 2)
                # ct store with bf16->fp8 cast done by the DMA itself
                # (SWDGE-only feature): zero engine cost, fp8 HBM bytes
                nc.gpsimd.dma_start(
                    ctT.ap()[:, p * PTOK:(p + 1) * PTOK], ct_t[:])

                vt_big = vt_tiles[p]
                th_tile = outp.tile([128, EH, PTOK], BF16)
                for g in range(2):
                    # PSUM -> SBUF bf16 on the Scalar engine (one FD-1024
                    # op per group) so the gate runs in DVE 2x mode
                    mm_sb = mmsb.tile([128, EH, GTOK], BF16,
                                      name=f"mm_sb_{p}_{g}")
                    nc.scalar.copy(mm_sb[:], mm_ps[g][:])
                    nc.vector.tensor_tensor(
                        th_tile[:, :, g * GTOK:(g + 1) * GTOK],
                        vt_big[:, :, g * GTOK:(g + 1) * GTOK],
                        mm_sb[:],
                        mybir.AluOpType.mult,
                    )
                    if p == NPAIR - 1:
                        # endgame: store per group so the final drain is
                        # half-sized and starts right after its gate
                        lo = p * PTOK + g * GTOK
                        nc.sync.dma_start(
                            th_view[:, :, lo:lo + GTOK],
                            th_tile[:, :, g * GTOK:(g + 1) * GTOK])
                if p < NPAIR - 1:
                    nc.sync.dma_start(
                        th_view[:, :, p * PTOK:(p + 1) * PTOK], th_tile[:])

    nc.compile()
    return nc


def _get_nc():
    global _NC
    if _NC is None:
        _NC = _build_nc()
    return _NC


def _host_prep(vt, rgap, sgap, pcount, W):
    import ml_dtypes
    bf16 = ml_dtypes.bfloat16
    vt = np.asarray(vt, dtype=np.float32)
    rgap = np.asarray(rgap)
    sgap = np.asarray(sgap)
    pcount = np.asarray(pcount)
    W = np.asarray(W, dtype=np.float32)
    wt = np.ascontiguousarray(W.T).astype(bf16)     # [96, 256]
    in_maps = []
    for m in range(N_CORES):
        sl = slice(m * BPC, (m + 1) * BPC)
        # token t = p*64 + i maps to column tau = i*128 + p
        idxs = np.stack(
            [rgap[sl].reshape(NTOK),
             sgap[sl].reshape(NTOK) + 32,
             pcount[sl].reshape(NTOK) + 64], axis=0)          # [3, t]
        idxt = np.ascontiguousarray(
            idxs.reshape(3, 128, NCH).transpose(0, 2, 1).reshape(3, NTOK)
        ).astype(bf16)
        vtT = np.ascontiguousarray(
            vt[sl].reshape(128, NCH, E).transpose(2, 1, 0).reshape(E, NTOK)
        ).astype(bf16)
        in_maps.append({"vtT": vtT, "idxt": idxt, "wt": wt})
    return in_maps


def kernel(vt, rgap, sgap, pcount, W, _trace=False, _tmpdir=None):
    nc = _get_nc()
    in_maps = _host_prep(vt, rgap, sgap, pcount, W)
    res = run_bass_kernel_spmd(
        nc, in_maps, list(range(N_CORES)),
        trace=_trace, **({"tmpdir": _tmpdir} if _tmpdir else {}),
    )
    full = np.empty((B, S, E + NTOT), dtype=np.float32)
    for m in range(N_CORES):
        sl = slice(m * BPC, (m + 1) * BPC)
        view = full[sl].reshape(NTOK, E + NTOT)
        thetaT = np.asarray(res.results[m]["thetaT"]).astype(np.float32)
        ct8 = np.asarray(res.results[m]["ctT"]).astype(np.float32)
        view[:, :E] = thetaT.reshape(E, NCH, 128).transpose(2, 1, 0) \
                            .reshape(NTOK, E)
        view[:, E:] = ct8.reshape(NTOT, NCH, 128).transpose(2, 1, 0) \
                         .reshape(NTOK, NTOT)
    if _trace:
        return full, res
    return full
